# revision 29
# baseline (speedup 1.0000x reference)
"""Multi-head self-attention (B=8, S=1024, D=1024, H=16) on 8 TRN2 NeuronCores.

Sharding: data-parallel over batch — one batch element per core, weights
replicated; no collectives needed.

Per-core kernel runs attention in a transposed layout so the only on-chip
transpose is X^T (64 PE transposes):
  X^T [d, s]            PE transpose of the input
  Q^T, K^T [c, s]       = W_{q,k}.T @ X^T   (channel tiles on partitions)
  V [s, c]              natural orientation, with a ones column per head
  scores^T [k, q]       = K_h @ Q_h^T       (contraction over head dim = 64)
  P^T = exp(scores^T)   no max subtraction (|scores| <~ 6 by construction)
  num^T [65, q]         = V'_h.T @ P^T      row 64 = softmax denominator
  attnout^T [c, q]      = num^T[0:64] * (1/denom)  (gpsimd partition_broadcast)
  out [s, d]            = attnout^T.T @ W_proj + b_proj
Matmuls use float32r (fast fp32 PE path, ~1.7e-3 component precision); the
softmax probabilities and V run in bf16 (exp writes bf16 ~20% faster on ACT,
which is the attention-phase bottleneck engine). Even/odd head pairs are
emitted back-to-back so their K=64 score matmuls overlap in disjoint PE row
groups (measured 4x). End-to-end error vs the fp32 reference: ~2e-3.
"""

from contextlib import ExitStack

import numpy as np

import concourse.mybir as mybir
import concourse.tile as tile
from concourse import bacc
from concourse.bass_utils import run_bass_kernel_spmd
from concourse.masks import make_identity

S = 1024  # sequence length (per core batch element)
D = 1024  # embed dim
H = 16  # heads
HD = 64  # head dim
P = 128  # partitions
NCORES = 8
NG = 4  # head groups (4 heads / 256 channels each)
GC = 256  # channels per group
SCALE = 1.0 / 8.0  # 1/sqrt(HD)

F32 = mybir.dt.float32
F32R = mybir.dt.float32r
BF16 = mybir.dt.bfloat16
AF = mybir.ActivationFunctionType


def make_pools(ctx, tc):
    return {
        "const": ctx.enter_context(tc.tile_pool(name="const", bufs=1)),
        "xtp": ctx.enter_context(tc.tile_pool(name="xtp", bufs=1)),
        "xinp": ctx.enter_context(tc.tile_pool(name="xinp", bufs=3)),
        "wblkp": ctx.enter_context(tc.tile_pool(name="wblkp", bufs=4)),
        "qkp": ctx.enter_context(tc.tile_pool(name="qkp", bufs=2)),
        "vgp": ctx.enter_context(tc.tile_pool(name="vgp", bufs=2)),
        "ptp": ctx.enter_context(tc.tile_pool(name="ptp", bufs=2)),
        "wpp": ctx.enter_context(tc.tile_pool(name="wpp", bufs=1)),
        "smp": ctx.enter_context(tc.tile_pool(name="smp", bufs=4)),
        "ps": ctx.enter_context(tc.tile_pool(name="ps", bufs=2, space="PSUM")),
    }


def emit_mha(pools, tc, out, x, wqkv, bqkv, wproj, bproj, two_ko=True, gp_bcast=True, pt_bf16=True):
    nc = tc.nc

    const = pools["const"]
    xt_pool = pools["xtp"]
    xin_pool = pools["xinp"]
    wblk_pool = pools["wblkp"]
    qk_pool = pools["qkp"]
    vg_pool = pools["vgp"]
    pt_pool = pools["ptp"]
    wp_pool = pools["wpp"]
    sm_pool = pools["smp"]
    ps = pools["ps"]

    # ---- start the big input DMAs first (X tiles; W streams follow via
    # the group loop) so the DMA queues ramp while constants are built ----
    xins = []
    for so in range(8):
        xin = xin_pool.tile([P, D], F32R, tag="xin", bufs=3, name="xin")
        nc.sync.dma_start(xin, x[so * P : (so + 1) * P, :])
        xins.append(xin)

    # ---- constants / biases ----
    # f32r tiles cannot be memset directly (ISA restriction); build f32
    # versions and DVE-copy, which performs the f32 -> f32r rounding.
    identf = const.tile([P, P], F32, name="identf")
    make_identity(nc, identf)
    ident = const.tile([P, P], F32R, name="ident")
    nc.vector.tensor_copy(ident, identf)
    onesf = const.tile([P, P], F32, name="onesf")
    nc.vector.memset(onesf, 1.0)
    ones128 = const.tile([1, P], F32R, name="ones128")
    nc.vector.tensor_copy(ones128, onesf[0:1, :])
    ones64 = const.tile([1, HD], F32R, name="ones64")
    nc.vector.tensor_copy(ones64, onesf[0:1, 0:HD])

    # b_qkv striped per-partition: b_sb[p, col] = b_qkv[col*128 + p]
    b_sb = const.tile([P, 24], F32, name="b_sb")
    nc.sync.dma_start(b_sb, bqkv.rearrange("(col p) -> p col", p=P))
    bq_s = const.tile([P, 8], F32, name="bq_s")  # pre-scaled Q bias
    nc.vector.tensor_scalar_mul(bq_s, b_sb[:, 0:8], SCALE)

    # V and proj biases broadcast to [128, D] via ones-matmul
    bvrow = xin_pool.tile([1, D], F32R, tag="xin", name="bvrow")
    nc.gpsimd.dma_start(bvrow, bqkv[2 * D : 3 * D].rearrange("(a c) -> a c", a=1))
    bprow = xin_pool.tile([1, D], F32R, tag="xin", name="bprow")
    nc.gpsimd.dma_start(bprow, bproj.rearrange("(a c) -> a c", a=1))
    bvb = const.tile([P, D], F32, name="bvb")
    bpb = const.tile([P, D], F32, name="bpb")
    for row, dst in ((bvrow, bvb), (bprow, bpb)):
        for ch in range(2):
            psb = ps.tile([P, 512], F32, tag="mm", bufs=2, name="psb")
            nc.tensor.matmul(
                psb, lhsT=ones128, rhs=row[:, ch * 512 : (ch + 1) * 512],
                start=True, stop=True,
            )
            nc.vector.tensor_copy(dst[:, ch * 512 : (ch + 1) * 512], psb)

    # ---- X^T ----
    xt = xt_pool.tile([P, 8, S], F32R, tag="xt", name="xt")
    for so in range(8):
        xin = xins[so]
        for do in range(8):
            pst = ps.tile([P, P], F32R, tag="sc", bufs=2, name="pst")
            nc.tensor.transpose(pst, xin[:, do * P : (do + 1) * P], ident)
            nc.vector.tensor_copy(xt[:, do, so * P : (so + 1) * P], pst)

    attnt = xt_pool.tile([P, 8, S], F32R, tag="attnt", name="attnt")

    # ---- per head-group: QKV projection then attention ----
    pv_dt = BF16 if pt_bf16 else F32R
    for g in range(4):
        wq = wblk_pool.tile([P, 8, GC], F32R, tag="wblk", name="wq")
        nc.sync.dma_start(
            wq, wqkv[:, g * GC : (g + 1) * GC].rearrange("(ko p) c -> p ko c", p=P)
        )
        wk = wblk_pool.tile([P, 8, GC], F32R, tag="wblk", name="wk")
        nc.sync.dma_start(
            wk, wqkv[:, D + g * GC : D + (g + 1) * GC].rearrange(
                "(ko p) c -> p ko c", p=P
            )
        )
        wv = wblk_pool.tile([P, 8, GC], F32R, tag="wblk", name="wv")
        nc.sync.dma_start(
            wv, wqkv[:, 2 * D + g * GC : 2 * D + (g + 1) * GC].rearrange(
                "(ko p) c -> p ko c", p=P
            )
        )

        qt = qk_pool.tile([P, 2, S], F32R, tag="qt", name="qt")
        kt = qk_pool.tile([P, 2, S], F32R, tag="kt", name="kt")
        for cb in range(2):
            for qch in range(2):
                sl = slice(qch * 512, (qch + 1) * 512)
                psq = ps.tile([P, 512], F32, tag="mm", bufs=2, name="psq")
                for ko in range(8):
                    nc.tensor.matmul(
                        psq,
                        lhsT=wq[:, ko, cb * P : (cb + 1) * P],
                        rhs=xt[:, ko, sl],
                        start=(ko == 0),
                        stop=(ko == 7),
                    )
                nc.vector.tensor_scalar(
                    qt[:, cb, sl], psq,
                    SCALE, bq_s[:, 2 * g + cb : 2 * g + cb + 1],
                    mybir.AluOpType.mult, mybir.AluOpType.add,
                )
                psk = ps.tile([P, 512], F32, tag="mm", bufs=2, name="psk")
                for ko in range(8):
                    nc.tensor.matmul(
                        psk,
                        lhsT=wk[:, ko, cb * P : (cb + 1) * P],
                        rhs=xt[:, ko, sl],
                        start=(ko == 0),
                        stop=(ko == 7),
                    )
                nc.vector.tensor_scalar(
                    kt[:, cb, sl], psk,
                    b_sb[:, 8 + 2 * g + cb : 8 + 2 * g + cb + 1], None,
                    mybir.AluOpType.add,
                )

        # V for this group: [s, 4 heads x (64 + ones col)]
        vg = vg_pool.tile([P, 8, 4, HD + 1], pv_dt, tag="vg", name="vg")
        nc.vector.tensor_copy(
            vg[:, :, :, HD], onesf[:, 0:32].rearrange("p (a b) -> p a b", a=8)
        )
        for so in range(8):
            psv = ps.tile([P, GC], F32, tag="mm", bufs=2, name="psv")
            for ko in range(8):
                nc.tensor.matmul(
                    psv,
                    lhsT=xt[:, ko, so * P : (so + 1) * P],
                    rhs=wv[:, ko, :],
                    start=(ko == 0),
                    stop=(ko == 7),
                )
            nc.vector.tensor_add(
                out=vg[:, so, :, 0:HD],
                in0=psv.rearrange("p (h c) -> p h c", h=4),
                in1=bvb[:, g * GC : (g + 1) * GC].rearrange("p (h c) -> p h c", h=4),
            )

        # attention, processed as even/odd head pairs: the even head's channels
        # sit on partitions 0-63 and the odd head's on 64-127, so their K=64
        # score matmuls land in disjoint PE row groups (tile_position (0,0) /
        # (64,0)) and, emitted back-to-back, execute concurrently on hardware.
        for pp in range(2):
            heads = (2 * pp, 2 * pp + 1)  # even, odd within group
            for qch in range(2):
                qsl = slice(qch * 512, (qch + 1) * 512)
                pvs = [
                    ps.tile([P, 512], F32, tag="pv", bufs=2, name=f"pspv{i}")
                    for i in range(2)
                ]
                kw = 2 if two_ko else 1  # exp width in ko tiles
                for kp in range(8 // kw):
                    scs, pts = [], []
                    for i, hb in enumerate(heads):
                        scs.append(
                            ps.tile([P, kw, 512], F32, tag="sc", bufs=2, name="pssc")
                        )
                        pts.append(
                            pt_pool.tile(
                                [P, kw, 512], pv_dt, tag="pt",
                                bufs=3 if pt_bf16 else 2, name="pt",
                            )
                        )
                    for j in range(kw):
                        ko = kw * kp + j
                        for i, hb in enumerate(heads):
                            poff = (hb % 2) * HD
                            nc.tensor.matmul(
                                scs[i][:, j],
                                lhsT=kt[poff : poff + HD, pp, ko * P : (ko + 1) * P],
                                rhs=qt[poff : poff + HD, pp, qsl],
                                start=True,
                                stop=True,
                            )
                    for i in range(2):
                        nc.scalar.activation(pts[i], scs[i], AF.Exp)
                    for j in range(kw):
                        ko = kw * kp + j
                        for i, hb in enumerate(heads):
                            nc.tensor.matmul(
                                pvs[i][0 : HD + 1],
                                lhsT=vg[:, ko, hb],
                                rhs=pts[i][:, j],
                                start=(ko == 0),
                                stop=(ko == 7),
                            )
                for i, hb in enumerate(heads):
                    poff = (hb % 2) * HD
                    rec_dt = F32 if gp_bcast else F32R
                    rec = sm_pool.tile([1, 512], rec_dt, tag="rec", bufs=2, name="rec")
                    nc.vector.reciprocal(rec, pvs[i][HD : HD + 1, :])
                    recb = sm_pool.tile([HD, 512], F32, tag="recb", bufs=2, name="recb")
                    if gp_bcast:
                        nc.gpsimd.partition_broadcast(recb, rec)
                    else:
                        psbc = ps.tile([HD, 512], F32, tag="bc", bufs=1, name="psbc")
                        nc.tensor.matmul(
                            psbc, lhsT=ones64, rhs=rec, start=True, stop=True
                        )
                        nc.vector.tensor_copy(recb, psbc)
                    nc.vector.tensor_mul(
                        out=attnt[poff : poff + HD, 2 * g + pp, qsl],
                        in0=pvs[i][0:HD, :],
                        in1=recb,
                    )

    # ---- output projection (w_proj streamed in two 512-column chunks) ----
    for ch in range(2):
        sl = slice(ch * 512, (ch + 1) * 512)
        wp = wp_pool.tile([P, 8, 512], F32R, tag="wp", name="wp")
        nc.sync.dma_start(wp, wproj[:, sl].rearrange("(ko p) c -> p ko c", p=P))
        for so in range(8):
            psp = ps.tile([P, 512], F32, tag="mm", bufs=2, name="psp")
            for ko in range(8):
                nc.tensor.matmul(
                    psp,
                    lhsT=attnt[:, ko, so * P : (so + 1) * P],
                    rhs=wp[:, ko, :],
                    start=(ko == 0),
                    stop=(ko == 7),
                )
            ot = sm_pool.tile([P, 512], F32, tag="ot", bufs=3, name="ot")
            nc.vector.tensor_add(out=ot, in0=psp, in1=bpb[:, sl])
            nc.sync.dma_start(out[so * P : (so + 1) * P, sl], ot)


def build_nc(repeat=1, two_ko=True, gp_bcast=True, pt_bf16=True):
    nc = bacc.Bacc("TRN2", target_bir_lowering=False, debug=False, num_devices=NCORES)
    x = nc.dram_tensor("query", [S, D], F32R, kind="ExternalInput").ap()
    wqkv = nc.dram_tensor("w_qkv", [D, 3 * D], F32R, kind="ExternalInput").ap()
    bqkv = nc.dram_tensor("b_qkv", [3 * D], F32, kind="ExternalInput").ap()
    wproj = nc.dram_tensor("w_proj", [D, D], F32R, kind="ExternalInput").ap()
    bproj = nc.dram_tensor("b_proj", [D], F32, kind="ExternalInput").ap()
    out = nc.dram_tensor("out", [S, D], F32, kind="ExternalOutput").ap()
    with (
        tile.TileContext(nc) as tc,
        ExitStack() as ctx,
        nc.allow_low_precision(reason="float32r matmul pipeline (~1e-3)"),
    ):
        pools = make_pools(ctx, tc)
        for _ in range(repeat):
            emit_mha(
                pools, tc, out, x, wqkv, bqkv, wproj, bproj,
                two_ko=two_ko, gp_bcast=gp_bcast, pt_bf16=pt_bf16,
            )
    nc.compile()
    return nc


_NC_CACHE = None


def _get_nc():
    global _NC_CACHE
    if _NC_CACHE is None:
        _NC_CACHE = build_nc()
    return _NC_CACHE


def make_in_maps(query, w_qkv, b_qkv, w_proj, b_proj):
    f = np.float32
    shared = {
        "w_qkv": np.ascontiguousarray(w_qkv, dtype=f),
        "b_qkv": np.ascontiguousarray(b_qkv, dtype=f),
        "w_proj": np.ascontiguousarray(w_proj, dtype=f),
        "b_proj": np.ascontiguousarray(b_proj, dtype=f),
    }
    return [
        {"query": np.ascontiguousarray(query[i], dtype=f), **shared}
        for i in range(NCORES)
    ]


def kernel(query, w_qkv, b_qkv, w_proj, b_proj):
    nc = _get_nc()
    in_maps = make_in_maps(query, w_qkv, b_qkv, w_proj, b_proj)
    res = run_bass_kernel_spmd(nc, in_maps, core_ids=list(range(NCORES)))
    return np.stack([res.results[i]["out"] for i in range(NCORES)]).astype(np.float32)


# revision 31
# speedup vs baseline: 1.0927x; 1.0927x over previous
"""Multi-head self-attention (B=8, S=1024, D=1024, H=16) on 8 TRN2 NeuronCores.

Sharding: data-parallel over batch — one batch element per core, weights
replicated; no collectives needed.

Per-core kernel runs attention in a transposed layout so the only on-chip
transpose is X^T (64 PE transposes):
  X^T [d, s]            PE transpose of the input
  Q^T, K^T [c, s]       = W_{q,k}.T @ X^T   (channel tiles on partitions)
  V [s, c]              natural orientation, with a ones column per head
  scores^T [k, q]       = K_h @ Q_h^T       (contraction over head dim = 64)
  P^T = exp(scores^T)   no max subtraction (|scores| <~ 6 by construction)
  num^T [65, q]         = V'_h.T @ P^T      row 64 = softmax denominator
  attnout^T [c, q]      = num^T[0:64] * (1/denom)  (gpsimd partition_broadcast)
  out [s, d]            = attnout^T.T @ W_proj + b_proj
Matmuls use float32r (fast fp32 PE path, ~1.7e-3 component precision); the
softmax probabilities and V run in bf16 (exp writes bf16 ~20% faster on ACT,
which is the attention-phase bottleneck engine). Even/odd head pairs are
emitted back-to-back so their K=64 score matmuls overlap in disjoint PE row
groups (measured 4x). End-to-end error vs the fp32 reference: ~2e-3.
"""

from contextlib import ExitStack

import numpy as np

import concourse.mybir as mybir
import concourse.tile as tile
from concourse import bacc
from concourse.bass_utils import run_bass_kernel_spmd
from concourse.masks import make_identity

S = 1024  # sequence length (per core batch element)
D = 1024  # embed dim
H = 16  # heads
HD = 64  # head dim
P = 128  # partitions
NCORES = 8
NG = 4  # head groups (4 heads / 256 channels each)
GC = 256  # channels per group
SCALE = 1.0 / 8.0  # 1/sqrt(HD)

F32 = mybir.dt.float32
F32R = mybir.dt.float32r
BF16 = mybir.dt.bfloat16
AF = mybir.ActivationFunctionType


def make_pools(ctx, tc):
    return {
        "const": ctx.enter_context(tc.tile_pool(name="const", bufs=1)),
        "xtp": ctx.enter_context(tc.tile_pool(name="xtp", bufs=1)),
        "xinp": ctx.enter_context(tc.tile_pool(name="xinp", bufs=3)),
        "wblkp": ctx.enter_context(tc.tile_pool(name="wblkp", bufs=4)),
        "qkp": ctx.enter_context(tc.tile_pool(name="qkp", bufs=2)),
        "vgp": ctx.enter_context(tc.tile_pool(name="vgp", bufs=2)),
        "ptp": ctx.enter_context(tc.tile_pool(name="ptp", bufs=2)),
        "wpp": ctx.enter_context(tc.tile_pool(name="wpp", bufs=1)),
        "smp": ctx.enter_context(tc.tile_pool(name="smp", bufs=4)),
        "ps": ctx.enter_context(tc.tile_pool(name="ps", bufs=2, space="PSUM")),
    }


def emit_mha(pools, tc, out, x, wqkv, bqkv, wproj, bproj, two_ko=True, gp_bcast=True, pt_bf16=True):
    nc = tc.nc

    const = pools["const"]
    xt_pool = pools["xtp"]
    xin_pool = pools["xinp"]
    wblk_pool = pools["wblkp"]
    qk_pool = pools["qkp"]
    vg_pool = pools["vgp"]
    pt_pool = pools["ptp"]
    wp_pool = pools["wpp"]
    sm_pool = pools["smp"]
    ps = pools["ps"]

    # ---- start the big input DMAs first (X tiles; W streams follow via
    # the group loop) so the DMA queues ramp while constants are built ----
    xins = []
    for so in range(8):
        xin = xin_pool.tile([P, D], F32R, tag="xin", bufs=3, name="xin")
        nc.sync.dma_start(xin, x[so * P : (so + 1) * P, :])
        xins.append(xin)

    # ---- constants / biases ----
    # f32r tiles cannot be memset directly (ISA restriction); build f32
    # versions and DVE-copy, which performs the f32 -> f32r rounding.
    identf = const.tile([P, P], F32, name="identf")
    make_identity(nc, identf)
    ident = const.tile([P, P], F32R, name="ident")
    nc.vector.tensor_copy(ident, identf)
    onesf = const.tile([P, P], F32, name="onesf")
    nc.vector.memset(onesf, 1.0)
    ones128 = const.tile([1, P], F32R, name="ones128")
    nc.vector.tensor_copy(ones128, onesf[0:1, :])
    ones64 = const.tile([1, HD], F32R, name="ones64")
    nc.vector.tensor_copy(ones64, onesf[0:1, 0:HD])

    # b_qkv striped per-partition: b_sb[p, col] = b_qkv[col*128 + p]
    b_sb = const.tile([P, 24], F32, name="b_sb")
    nc.sync.dma_start(b_sb, bqkv.rearrange("(col p) -> p col", p=P))
    bq_s = const.tile([P, 8], F32, name="bq_s")  # pre-scaled Q bias
    nc.vector.tensor_scalar_mul(bq_s, b_sb[:, 0:8], SCALE)

    # V and proj biases broadcast to [128, D] via ones-matmul
    bvrow = xin_pool.tile([1, D], F32R, tag="xin", name="bvrow")
    nc.gpsimd.dma_start(bvrow, bqkv[2 * D : 3 * D].rearrange("(a c) -> a c", a=1))
    bprow = xin_pool.tile([1, D], F32R, tag="xin", name="bprow")
    nc.gpsimd.dma_start(bprow, bproj.rearrange("(a c) -> a c", a=1))
    bvb = const.tile([P, D], F32, name="bvb")
    bpb = const.tile([P, D], F32, name="bpb")
    for row, dst in ((bvrow, bvb), (bprow, bpb)):
        for ch in range(2):
            psb = ps.tile([P, 512], F32, tag="mm", bufs=2, name="psb")
            nc.tensor.matmul(
                psb, lhsT=ones128, rhs=row[:, ch * 512 : (ch + 1) * 512],
                start=True, stop=True,
            )
            nc.vector.tensor_copy(dst[:, ch * 512 : (ch + 1) * 512], psb)

    # ---- X^T, split into two half-sequence tiles so early QKV matmuls
    # depend only on the first 32 transposes, not all 64 ----
    xth = [
        xt_pool.tile([P, 8, S // 2], F32R, tag=f"xt{h}", name=f"xt{h}")
        for h in range(2)
    ]
    for so in range(8):
        xin = xins[so]
        for do in range(8):
            pst = ps.tile([P, P], F32R, tag="sc", bufs=2, name="pst")
            nc.tensor.transpose(pst, xin[:, do * P : (do + 1) * P], ident)
            nc.vector.tensor_copy(
                xth[so // 4][:, do, (so % 4) * P : (so % 4 + 1) * P], pst
            )

    def xt_slice(ko, s0, s1):
        # contiguous [s0:s1) slice of X^T row-block ko; must stay in one half
        h = s0 // 512
        assert (s1 - 1) // 512 == h
        return xth[h][:, ko, s0 - h * 512 : s1 - h * 512]

    attnt = xt_pool.tile([P, 8, S], F32R, tag="attnt", name="attnt")

    # ---- per head-group: QKV projection then attention ----
    pv_dt = BF16 if pt_bf16 else F32R
    for g in range(4):
        wq = wblk_pool.tile([P, 8, GC], F32R, tag="wblk", name="wq")
        nc.sync.dma_start(
            wq, wqkv[:, g * GC : (g + 1) * GC].rearrange("(ko p) c -> p ko c", p=P)
        )
        wk = wblk_pool.tile([P, 8, GC], F32R, tag="wblk", name="wk")
        nc.sync.dma_start(
            wk, wqkv[:, D + g * GC : D + (g + 1) * GC].rearrange(
                "(ko p) c -> p ko c", p=P
            )
        )
        wv = wblk_pool.tile([P, 8, GC], F32R, tag="wblk", name="wv")
        nc.sync.dma_start(
            wv, wqkv[:, 2 * D + g * GC : 2 * D + (g + 1) * GC].rearrange(
                "(ko p) c -> p ko c", p=P
            )
        )

        qt = qk_pool.tile([P, 2, S], F32R, tag="qt", name="qt")
        kt = qk_pool.tile([P, 2, S], F32R, tag="kt", name="kt")
        for cb in range(2):
            for qch in range(2):
                sl = slice(qch * 512, (qch + 1) * 512)
                psq = ps.tile([P, 512], F32, tag="mm", bufs=2, name="psq")
                for ko in range(8):
                    nc.tensor.matmul(
                        psq,
                        lhsT=wq[:, ko, cb * P : (cb + 1) * P],
                        rhs=xt_slice(ko, qch * 512, (qch + 1) * 512),
                        start=(ko == 0),
                        stop=(ko == 7),
                    )
                nc.vector.tensor_scalar(
                    qt[:, cb, sl], psq,
                    SCALE, bq_s[:, 2 * g + cb : 2 * g + cb + 1],
                    mybir.AluOpType.mult, mybir.AluOpType.add,
                )
                psk = ps.tile([P, 512], F32, tag="mm", bufs=2, name="psk")
                for ko in range(8):
                    nc.tensor.matmul(
                        psk,
                        lhsT=wk[:, ko, cb * P : (cb + 1) * P],
                        rhs=xt_slice(ko, qch * 512, (qch + 1) * 512),
                        start=(ko == 0),
                        stop=(ko == 7),
                    )
                nc.vector.tensor_scalar(
                    kt[:, cb, sl], psk,
                    b_sb[:, 8 + 2 * g + cb : 8 + 2 * g + cb + 1], None,
                    mybir.AluOpType.add,
                )

        # V for this group: [s, 4 heads x (64 + ones col)]
        vg = vg_pool.tile([P, 8, 4, HD + 1], pv_dt, tag="vg", name="vg")
        nc.vector.tensor_copy(
            vg[:, :, :, HD], onesf[:, 0:32].rearrange("p (a b) -> p a b", a=8)
        )
        for so in range(8):
            psv = ps.tile([P, GC], F32, tag="mm", bufs=2, name="psv")
            for ko in range(8):
                nc.tensor.matmul(
                    psv,
                    lhsT=xt_slice(ko, so * P, (so + 1) * P),
                    rhs=wv[:, ko, :],
                    start=(ko == 0),
                    stop=(ko == 7),
                )
            nc.vector.tensor_add(
                out=vg[:, so, :, 0:HD],
                in0=psv.rearrange("p (h c) -> p h c", h=4),
                in1=bvb[:, g * GC : (g + 1) * GC].rearrange("p (h c) -> p h c", h=4),
            )

        # attention, processed as even/odd head pairs: the even head's channels
        # sit on partitions 0-63 and the odd head's on 64-127, so their K=64
        # score matmuls land in disjoint PE row groups (tile_position (0,0) /
        # (64,0)) and, emitted back-to-back, execute concurrently on hardware.
        for pp in range(2):
            heads = (2 * pp, 2 * pp + 1)  # even, odd within group
            for qch in range(2):
                qsl = slice(qch * 512, (qch + 1) * 512)
                pvs = [
                    ps.tile([P, 512], F32, tag="pv", bufs=2, name=f"pspv{i}")
                    for i in range(2)
                ]
                kw = 2 if two_ko else 1  # exp width in ko tiles
                for kp in range(8 // kw):
                    scs, pts = [], []
                    for i, hb in enumerate(heads):
                        scs.append(
                            ps.tile([P, kw, 512], F32, tag="sc", bufs=2, name="pssc")
                        )
                        pts.append(
                            pt_pool.tile(
                                [P, kw, 512], pv_dt, tag="pt",
                                bufs=4 if pt_bf16 else 2, name="pt",
                            )
                        )
                    for j in range(kw):
                        ko = kw * kp + j
                        for i, hb in enumerate(heads):
                            poff = (hb % 2) * HD
                            nc.tensor.matmul(
                                scs[i][:, j],
                                lhsT=kt[poff : poff + HD, pp, ko * P : (ko + 1) * P],
                                rhs=qt[poff : poff + HD, pp, qsl],
                                start=True,
                                stop=True,
                            )
                    for i in range(2):
                        nc.scalar.activation(pts[i], scs[i], AF.Exp)
                    for j in range(kw):
                        ko = kw * kp + j
                        for i, hb in enumerate(heads):
                            nc.tensor.matmul(
                                pvs[i][0 : HD + 1],
                                lhsT=vg[:, ko, hb],
                                rhs=pts[i][:, j],
                                start=(ko == 0),
                                stop=(ko == 7),
                            )
                for i, hb in enumerate(heads):
                    poff = (hb % 2) * HD
                    rec_dt = F32 if gp_bcast else F32R
                    rec = sm_pool.tile([1, 512], rec_dt, tag="rec", bufs=2, name="rec")
                    nc.vector.reciprocal(rec, pvs[i][HD : HD + 1, :])
                    recb = sm_pool.tile([HD, 512], F32, tag="recb", bufs=2, name="recb")
                    if gp_bcast:
                        nc.gpsimd.partition_broadcast(recb, rec)
                    else:
                        psbc = ps.tile([HD, 512], F32, tag="bc", bufs=1, name="psbc")
                        nc.tensor.matmul(
                            psbc, lhsT=ones64, rhs=rec, start=True, stop=True
                        )
                        nc.vector.tensor_copy(recb, psbc)
                    nc.vector.tensor_mul(
                        out=attnt[poff : poff + HD, 2 * g + pp, qsl],
                        in0=pvs[i][0:HD, :],
                        in1=recb,
                    )

    # ---- output projection (w_proj streamed in two 512-column chunks) ----
    for ch in range(2):
        sl = slice(ch * 512, (ch + 1) * 512)
        wp = wp_pool.tile([P, 8, 512], F32R, tag="wp", name="wp")
        nc.sync.dma_start(wp, wproj[:, sl].rearrange("(ko p) c -> p ko c", p=P))
        for so in range(8):
            psp = ps.tile([P, 512], F32, tag="mm", bufs=2, name="psp")
            for ko in range(8):
                nc.tensor.matmul(
                    psp,
                    lhsT=attnt[:, ko, so * P : (so + 1) * P],
                    rhs=wp[:, ko, :],
                    start=(ko == 0),
                    stop=(ko == 7),
                )
            ot = sm_pool.tile([P, 512], F32, tag="ot", bufs=3, name="ot")
            nc.vector.tensor_add(out=ot, in0=psp, in1=bpb[:, sl])
            nc.sync.dma_start(out[so * P : (so + 1) * P, sl], ot)


def build_nc(repeat=1, two_ko=True, gp_bcast=True, pt_bf16=True):
    nc = bacc.Bacc("TRN2", target_bir_lowering=False, debug=False, num_devices=NCORES)
    x = nc.dram_tensor("query", [S, D], F32R, kind="ExternalInput").ap()
    wqkv = nc.dram_tensor("w_qkv", [D, 3 * D], F32R, kind="ExternalInput").ap()
    bqkv = nc.dram_tensor("b_qkv", [3 * D], F32, kind="ExternalInput").ap()
    wproj = nc.dram_tensor("w_proj", [D, D], F32R, kind="ExternalInput").ap()
    bproj = nc.dram_tensor("b_proj", [D], F32, kind="ExternalInput").ap()
    out = nc.dram_tensor("out", [S, D], F32, kind="ExternalOutput").ap()
    with (
        tile.TileContext(nc) as tc,
        ExitStack() as ctx,
        nc.allow_low_precision(reason="float32r matmul pipeline (~1e-3)"),
    ):
        pools = make_pools(ctx, tc)
        for _ in range(repeat):
            emit_mha(
                pools, tc, out, x, wqkv, bqkv, wproj, bproj,
                two_ko=two_ko, gp_bcast=gp_bcast, pt_bf16=pt_bf16,
            )
    nc.compile()
    return nc


_NC_CACHE = None


def _get_nc():
    global _NC_CACHE
    if _NC_CACHE is None:
        _NC_CACHE = build_nc()
    return _NC_CACHE


def make_in_maps(query, w_qkv, b_qkv, w_proj, b_proj):
    f = np.float32
    shared = {
        "w_qkv": np.ascontiguousarray(w_qkv, dtype=f),
        "b_qkv": np.ascontiguousarray(b_qkv, dtype=f),
        "w_proj": np.ascontiguousarray(w_proj, dtype=f),
        "b_proj": np.ascontiguousarray(b_proj, dtype=f),
    }
    return [
        {"query": np.ascontiguousarray(query[i], dtype=f), **shared}
        for i in range(NCORES)
    ]


def kernel(query, w_qkv, b_qkv, w_proj, b_proj):
    nc = _get_nc()
    in_maps = make_in_maps(query, w_qkv, b_qkv, w_proj, b_proj)
    res = run_bass_kernel_spmd(nc, in_maps, core_ids=list(range(NCORES)))
    return np.stack([res.results[i]["out"] for i in range(NCORES)]).astype(np.float32)
